# revision 21
# baseline (speedup 1.0000x reference)
"""Trainium2 Bass kernel for naive causal MHA (dense transformer block).

Problem: x[2, 2048, 1024], per-head QKV (16 heads, head_dim 64), causal
softmax attention, concat heads, output projection.

Sharding (8 NeuronCores, tensor-parallel over heads):
  - core c computes QKV + attention for heads {2c, 2c+1} over both batches,
    in a transposed layout: scores are built as [keys, queries] so the
    softmax denominator comes from an extra ones-column in V and the
    attention output lands directly in the [head_dim, seq] layout the
    output projection needs as its stationary operand.
  - an 8-way AllToAll (bf16) reshards y from head-split to row-split,
  - each core computes a disjoint 512-row slice of y @ Wout + bout.

v2 vs baseline:
  - bf16 storage/matmul operands everywhere (fp32 PSUM accumulate);
    halves DMA + collective bytes, enables fast weight loads.
  - causal trimming: diagonal score blocks only compute/exp the valid
    column range; AV matmuls stream the valid subrange.
  - 2-deep software pipeline (scores for tb+2 issued before AV of tb)
    to hide exp latency and keep the PE HAM-warm (2.4 GHz).
  - softmax normalize: copy psum out early + reciprocal_approx_fast +
    gpsimd partition_broadcast (no DRAM round-trip, no psum stalls).
  - wout/bout loads deferred past the x loads (kills the startup stall).
"""

import contextlib
import ctypes
import sys
import types

import ml_dtypes
import numpy as np

import concourse.bacc as bacc
import concourse.mybir as mybir
import concourse.tile as tile
from concourse.bass import ds

N_CORES = 8
B = 2
S = 2048
D = 1024
HD = 64
N_HEADS = 16

BF = mybir.dt.bfloat16
F32 = mybir.dt.float32
F8 = mybir.dt.float8e4

SC = 512          # seq chunk (moving-operand width)
N_SC = S // SC    # 4
N_DC = D // 128   # 8 contraction chunks
N_SB = S // 128   # 16 seq 128-blocks

NPBF = ml_dtypes.bfloat16


def _build_program(dbg=False):
    nc = bacc.Bacc(
        "TRN2", target_bir_lowering=False, debug=False, num_devices=N_CORES
    )

    # xt[b, sc, dc, p, col] = x[b, sc*512+col, 128*dc+p]
    xt_d = nc.dram_tensor("xt", [B, N_SC, N_DC, 128, SC], BF, kind="ExternalInput").ap()
    wq_d = nc.dram_tensor("wq", [128, N_DC, 128], BF, kind="ExternalInput").ap()
    wk_d = nc.dram_tensor("wk", [128, N_DC, 128], BF, kind="ExternalInput").ap()
    wv_d = nc.dram_tensor("wv", [128, N_DC, 128], BF, kind="ExternalInput").ap()
    bq_d = nc.dram_tensor("bq", [128, 1], F32, kind="ExternalInput").ap()
    bk_d = nc.dram_tensor("bk", [128, 1], F32, kind="ExternalInput").ap()
    bv4_d = nc.dram_tensor("bv4", [1, 512], F32, kind="ExternalInput").ap()
    tri_d = nc.dram_tensor("tri", [128, 128], BF, kind="ExternalInput").ap()
    wout_d = nc.dram_tensor("wout", [128, N_DC, D], BF, kind="ExternalInput").ap()
    bout_d = nc.dram_tensor("bout", [1, D], F32, kind="ExternalInput").ap()
    out_d = nc.dram_tensor("out", [512, D], F32, kind="ExternalOutput").ap()

    y_part = nc.dram_tensor("y_part", [8, 128, SC], BF)
    y_all = nc.dram_tensor("y_all", [8, 128, SC], BF)
    z_bounce = nc.dram_tensor("z_bounce", [4, 1, SC], BF)
    if dbg:
        dbg_qT = nc.dram_tensor("dbg_qT", [B, 64, 2, S], BF, kind="ExternalOutput").ap()
        dbg_kT = nc.dram_tensor("dbg_kT", [B, 64, 2, S], BF, kind="ExternalOutput").ap()
        dbg_v = nc.dram_tensor("dbg_v", [B, 128, N_SB * 2 * 65], BF, kind="ExternalOutput").ap()
        dbg_z = nc.dram_tensor("dbg_z", [B, 4, 2, 1, SC], mybir.dt.float32, kind="ExternalOutput").ap()
        dbg_zaf = nc.dram_tensor("dbg_zaf", [B, 4, 2, 1, SC], mybir.dt.float32, kind="ExternalOutput").ap()
        dbg_ex = nc.dram_tensor("dbg_ex", [B, 4, 2, 128, SC], BF, kind="ExternalOutput").ap()
        dbg_yp = nc.dram_tensor("dbg_yp", [8, 128, SC], BF, kind="ExternalOutput").ap()

    with tile.TileContext(nc) as tc, contextlib.ExitStack() as ctx:
        const = ctx.enter_context(tc.tile_pool(name="const", bufs=1))
        xt_pool = ctx.enter_context(tc.tile_pool(name="xt", bufs=18))
        qk_pool = ctx.enter_context(tc.tile_pool(name="qk", bufs=2))
        v_pool = ctx.enter_context(tc.tile_pool(name="vp", bufs=2))
        exp_pool = ctx.enter_context(tc.tile_pool(name="expp", bufs=6))
        yn_pool = ctx.enter_context(tc.tile_pool(name="yn", bufs=3))
        z_pool = ctx.enter_context(tc.tile_pool(name="zp", bufs=2))
        yg_pool = ctx.enter_context(tc.tile_pool(name="yg", bufs=1))
        outs_pool = ctx.enter_context(tc.tile_pool(name="outs", bufs=3))
        psum = ctx.enter_context(tc.tile_pool(name="psum", bufs=1, space="PSUM"))

        # ---- early constants (needed by QKV/attention) ----
        # per-chunk loads parallelize across DMA queues (startup latency)
        wq_sb = const.tile([128, N_DC, 128], BF)
        wk_sb = const.tile([128, N_DC, 128], BF)
        wv_sb = const.tile([128, N_DC, 128], BF)
        for dc in range(N_DC):
            nc.sync.dma_start(out=wq_sb[:, dc, :], in_=wq_d[:, dc, :])
        for dc in range(N_DC):
            nc.sync.dma_start(out=wk_sb[:, dc, :], in_=wk_d[:, dc, :])
        for dc in range(N_DC):
            nc.sync.dma_start(out=wv_sb[:, dc, :], in_=wv_d[:, dc, :])
        bq_sb = const.tile([128, 1], F32)
        nc.sync.dma_start(out=bq_sb, in_=bq_d)
        bk_sb = const.tile([128, 1], F32)
        nc.sync.dma_start(out=bk_sb, in_=bk_d)
        bv4_bc = const.tile([128, 512], F32)
        nc.sync.dma_start(out=bv4_bc, in_=bv4_d.to_broadcast([128, 512]))
        tri_sb = const.tile([128, 128], BF)
        nc.sync.dma_start(out=tri_sb, in_=tri_d)

        for b in range(B):
            # ---- QKV projection for batch b (2 heads) ----
            # q/k in fp8e4: pre-fold [64(hd), head, S] then DMA partition-shift
            # into the DoubleRow fold [32, i, head, S] (hd = 32*i + p)
            qT = qk_pool.tile([64, 2, S], F8, tag="qT")
            kT = qk_pool.tile([64, 2, S], F8, tag="kT")
            qT8 = qk_pool.tile([32, 2, 2, S], F8, tag="qT8")
            kT8 = qk_pool.tile([32, 2, 2, S], F8, tag="kT8")
            v_sb = v_pool.tile([128, N_SB, 2, 65], BF)
            nc.vector.memset(v_sb[:, :, :, 64:65], 1.0)
            for sc in range(N_SC):
                xts = []
                for dc in range(N_DC):
                    xt = xt_pool.tile([128, SC], BF)
                    nc.sync.dma_start(out=xt, in_=xt_d[b, sc, dc])
                    xts.append(xt)
                psq = psum.tile([128, SC], F32, tag="qkv", bufs=1)
                for dc in range(N_DC):
                    nc.tensor.matmul(
                        psq, wq_sb[:, dc, :], xts[dc],
                        start=(dc == 0), stop=(dc == N_DC - 1),
                    )
                for h in range(2):
                    nc.vector.tensor_scalar_add(
                        out=qT[:, h, ds(sc * SC, SC)],
                        in0=psq[ds(64 * h, 64), :],
                        scalar1=bq_sb[ds(64 * h, 64), :],
                    )
                for i in range(2):
                    nc.sync.dma_start(
                        out=qT8[:, i, :, ds(sc * SC, SC)],
                        in_=qT[ds(32 * i, 32), :, ds(sc * SC, SC)],
                    )
                psk = psum.tile([128, SC], F32, tag="qkv", bufs=1)
                for dc in range(N_DC):
                    nc.tensor.matmul(
                        psk, wk_sb[:, dc, :], xts[dc],
                        start=(dc == 0), stop=(dc == N_DC - 1),
                    )
                for h in range(2):
                    nc.vector.tensor_scalar_add(
                        out=kT[:, h, ds(sc * SC, SC)],
                        in0=psk[ds(64 * h, 64), :],
                        scalar1=bk_sb[ds(64 * h, 64), :],
                    )
                for i in range(2):
                    nc.sync.dma_start(
                        out=kT8[:, i, :, ds(sc * SC, SC)],
                        in_=kT[ds(32 * i, 32), :, ds(sc * SC, SC)],
                    )
                psv = psum.tile([128, 4, 128], F32, tag="psv", bufs=1)
                for j4 in range(4):
                    for dc in range(N_DC):
                        nc.tensor.matmul(
                            psv[:, j4, :],
                            xts[dc][:, ds(j4 * 128, 128)],
                            wv_sb[:, dc, :],
                            start=(dc == 0), stop=(dc == N_DC - 1),
                            skip_group_check=True,
                        )
                nc.vector.tensor_add(
                    out=v_sb[:, ds(4 * sc, 4), :, 0:64],
                    in0=psv.rearrange("p j (h e) -> p j h e", h=2),
                    in1=bv4_bc.rearrange("p (j h e) -> p j h e", j=4, h=2),
                )

            # ---- attention for batch b ----
            for qc in range(N_SC):
                ntb = 4 * qc + 4
                psys = [
                    psum.tile([65, SC], F32, tag="psy", bufs=2, name=f"psy{h}")
                    for h in range(2)
                ]
                exs = {}

                def issue_score(tb, qc=qc, exs=exs):
                    j = tb - 4 * qc  # >= 0: diagonal block index
                    off = 128 * j if j >= 0 else 0
                    w = SC - off
                    for h in range(2):
                        pss = psum.tile([128, SC], F32, tag="pss", bufs=4)
                        nc.tensor.matmul(
                            pss[:, off:],
                            kT8[:, :, h, ds(tb * 128, 128)],
                            qT8[:, :, h, ds(qc * SC + off, w)],
                            start=True, stop=True,
                            perf_mode=mybir.MatmulPerfMode.DoubleRow,
                        )
                        ex = exp_pool.tile([128, SC], BF)
                        nc.scalar.activation(
                            out=ex[:, off:], in_=pss[:, off:],
                            func=mybir.ActivationFunctionType.Exp,
                            scale=0.125,
                        )
                        if j >= 0:
                            nc.vector.tensor_mul(
                                out=ex[:, ds(off, 128)],
                                in0=ex[:, ds(off, 128)],
                                in1=tri_sb,
                            )
                        if dbg and tb == 0:
                            nc.sync.dma_start(out=dbg_ex[b, qc, h], in_=ex)
                        exs[(tb, h)] = (ex, off)

                issue_score(0)
                if ntb > 1:
                    issue_score(1)
                for tb in range(ntb):
                    if tb + 2 < ntb:
                        issue_score(tb + 2)
                    for h in range(2):
                        ex, off = exs.pop((tb, h))
                        nc.tensor.matmul(
                            psys[h][:, off:],
                            v_sb[:, tb, h, :],
                            ex[:, off:],
                            start=(tb == 0), stop=(tb == ntb - 1),
                            skip_group_check=True,
                        )
                for h in range(2):
                    psy = psys[h]
                    yraw = yn_pool.tile([64, SC], BF, tag=f"yraw{h}")
                    nc.vector.tensor_copy(out=yraw, in_=psy[0:64, :])
                    zs = z_pool.tile([1, SC], F32, tag=f"zs{h}")
                    nc.vector.tensor_copy(out=zs, in_=psy[64:65, :])
                    zr = z_pool.tile([1, SC], F32, tag=f"zr{h}")
                    nc.vector.reciprocal_approx_fast(out=zr, in_=zs)
                    if dbg:
                        nc.sync.dma_start(out=dbg_z[b, qc, h], in_=zr)
                    zb = z_pool.tile([1, SC], BF, tag=f"zb{h}")
                    nc.vector.tensor_copy(out=zb, in_=zr)
                    zbb = z_pool.tile([64, SC], BF, tag=f"zbb{h}")
                    nc.gpsimd.partition_broadcast(zbb, zb)
                    yts = yn_pool.tile([64, SC], BF, tag=f"yts{h}")
                    nc.vector.tensor_mul(out=yts, in0=yraw, in1=zbb)
                    nc.sync.dma_start(
                        out=y_part.ap()[b * 4 + qc, ds(64 * h, 64), :], in_=yts
                    )

            if dbg:
                nc.sync.dma_start(out=dbg_qT[b], in_=qT)
                nc.sync.dma_start(out=dbg_kT[b], in_=kT)
                nc.sync.dma_start(
                    out=dbg_v[b], in_=v_sb.rearrange("p a b c -> p (a b c)")
                )

        if dbg:
            nc.sync.dma_start(out=dbg_yp, in_=y_part.ap())

        # ---- late constants (output projection) ----
        wout_sb = const.tile([128, N_DC, D], BF)
        nc.sync.dma_start(out=wout_sb, in_=wout_d)
        bout_bc = const.tile([128, D], F32)
        nc.sync.dma_start(out=bout_bc, in_=bout_d.to_broadcast([128, D]))

        # ---- reshard: head-split -> row-split ----
        nc.gpsimd.collective_compute(
            "AllToAll",
            mybir.AluOpType.bypass,
            replica_groups=[list(range(N_CORES))],
            ins=[y_part.ap()],
            outs=[y_all.ap()],
        )

        # ---- output projection for this core's 512 rows ----
        ygs = []
        for ec in range(8):
            yg = yg_pool.tile([128, SC], BF, tag=f"yg{ec}")
            nc.sync.dma_start(out=yg, in_=y_all.ap()[ec])
            ygs.append(yg)
        for sb in range(4):
            for ch in range(2):
                pso = psum.tile([128, SC], F32, tag="pss", bufs=4)
                for ec in range(8):
                    nc.tensor.matmul(
                        pso,
                        ygs[ec][:, ds(sb * 128, 128)],
                        wout_sb[:, ec, ds(ch * SC, SC)],
                        start=(ec == 0), stop=(ec == 7),
                    )
                ot = outs_pool.tile([128, SC], F32)
                nc.vector.tensor_add(
                    out=ot, in0=pso, in1=bout_bc[:, ds(ch * SC, SC)]
                )
                nc.sync.dma_start(
                    out=out_d[ds(sb * 128, 128), ds(ch * SC, SC)], in_=ot
                )

    nc.compile()
    return nc


_NC_CACHE = None


def _get_program():
    global _NC_CACHE
    if _NC_CACHE is None:
        _NC_CACHE = _build_program()
    return _NC_CACHE


def make_in_maps(x, Wqkv, bqkv, Wout, bout):
    x = np.asarray(x, dtype=np.float32)
    Wqkv = np.asarray(Wqkv, dtype=np.float32)
    bqkv = np.asarray(bqkv, dtype=np.float32)
    Wout = np.asarray(Wout, dtype=np.float32)
    bout = np.asarray(bout, dtype=np.float32)

    # xt[b, sc, dc, p, col] = x[b, sc*512+col, 128*dc+p]
    xt = np.ascontiguousarray(
        x.reshape(B, N_SC, SC, N_DC, 128).transpose(0, 1, 3, 4, 2)
    ).astype(NPBF)
    wout_t = np.ascontiguousarray(
        Wout.reshape(N_DC, 128, D).transpose(1, 0, 2)
    ).astype(NPBF)
    bout2 = np.ascontiguousarray(bout.reshape(1, D))
    tri = np.triu(np.ones((128, 128), dtype=np.float32)).astype(NPBF)

    def wslice(lo, hi, c):
        h0, h1 = 2 * c, 2 * c + 1
        w = np.concatenate([Wqkv[h0, :, lo:hi], Wqkv[h1, :, lo:hi]], axis=1)
        return np.ascontiguousarray(
            w.reshape(N_DC, 128, 128).transpose(1, 0, 2)
        ).astype(NPBF)

    in_maps = []
    for c in range(N_CORES):
        h0, h1 = 2 * c, 2 * c + 1
        bq = np.concatenate([bqkv[h0, 0:64], bqkv[h1, 0:64]]).reshape(128, 1)
        bk = np.concatenate([bqkv[h0, 64:128], bqkv[h1, 64:128]]).reshape(128, 1)
        bvp = np.concatenate([bqkv[h0, 128:192], bqkv[h1, 128:192]])
        bv4 = np.tile(bvp, 4).reshape(1, 512)
        in_maps.append(
            {
                "xt": xt,
                "wq": wslice(0, 64, c),
                "wk": wslice(64, 128, c),
                "wv": wslice(128, 192, c),
                "bq": np.ascontiguousarray(bq),
                "bk": np.ascontiguousarray(bk),
                "bv4": np.ascontiguousarray(bv4),
                "tri": tri,
                "wout": wout_t,
                "bout": bout2,
            }
        )
    return in_maps


def assemble(results):
    full = np.empty((N_CORES * 512, D), dtype=np.float32)
    for c in range(N_CORES):
        full[512 * c : 512 * (c + 1)] = results[c]["out"]
    return full.reshape(B, S, D)


def _install_ntff_hook():
    """The agent image's antenv lacks axon_hooks; provide it so
    run_bass_kernel_spmd(trace=True) can NTFF-profile via libaxon."""
    if "antenv.axon_hooks" in sys.modules:
        return
    so_path = "/opt/axon/libaxon_pjrt.so"
    try:
        lib = ctypes.CDLL(so_path)
        lib.axon_start_nrt_profile.argtypes = [
            ctypes.POINTER(ctypes.c_int64),
            ctypes.c_size_t,
        ]
        lib.axon_start_nrt_profile.restype = ctypes.c_int64
        lib.axon_stop_nrt_profile.argtypes = [ctypes.c_char_p]
        lib.axon_stop_nrt_profile.restype = ctypes.c_int64
    except (OSError, AttributeError):
        return

    @contextlib.contextmanager
    def _hook(output_dir, device_ids):
        import jax

        jax.devices()
        if device_ids:
            ids = (ctypes.c_int64 * len(device_ids))(*device_ids)
            rc = lib.axon_start_nrt_profile(ids, len(device_ids))
        else:
            rc = lib.axon_start_nrt_profile(None, 0)
        if rc != 0:
            raise RuntimeError(f"axon_start_nrt_profile rc={rc}")
        try:
            yield
        finally:
            n = lib.axon_stop_nrt_profile(str(output_dir).encode())
            if n < 0:
                raise RuntimeError(f"axon_stop_nrt_profile rc={n}")

    mod = types.ModuleType("antenv.axon_hooks")
    mod.get_axon_ntff_profile_hook = lambda: _hook
    mod.set_axon_ntff_profile_hook = lambda h: None
    sys.modules["antenv.axon_hooks"] = mod


def run(inputs, trace=False):
    """Run on the 8 NeuronCores. Returns (output, BassKernelResults)."""
    from concourse.bass_utils import run_bass_kernel_spmd

    if trace:
        _install_ntff_hook()
    nc = _get_program()
    in_maps = make_in_maps(**inputs)
    res = run_bass_kernel_spmd(
        nc, in_maps, core_ids=list(range(N_CORES)), trace=trace
    )
    return assemble(res.results), res


def kernel(x, Wqkv, bqkv, Wout, bout):
    out, _ = run(
        {"x": x, "Wqkv": Wqkv, "bqkv": bqkv, "Wout": Wout, "bout": bout},
        trace=False,
    )
    return out


# revision 29
# speedup vs baseline: 1.1541x; 1.1541x over previous
"""Trainium2 Bass kernel for naive causal MHA (dense transformer block).

Problem: x[2, 2048, 1024], per-head QKV (16 heads, head_dim 64), causal
softmax attention, concat heads, output projection.

Sharding (8 NeuronCores, tensor-parallel over heads):
  - core c computes QKV + attention for heads {2c, 2c+1} over both batches,
    in a transposed layout: scores are built as [keys, queries] so the
    softmax denominator comes from an extra ones-column in V and the
    attention output lands directly in the [head_dim, seq] layout the
    output projection needs as its stationary operand.
  - an 8-way AllToAll (bf16) reshards y from head-split to row-split,
  - each core computes a disjoint 512-row slice of y @ Wout + bout.

v2 vs baseline:
  - bf16 storage/matmul operands everywhere (fp32 PSUM accumulate);
    halves DMA + collective bytes, enables fast weight loads.
  - causal trimming: diagonal score blocks only compute/exp the valid
    column range; AV matmuls stream the valid subrange.
  - 2-deep software pipeline (scores for tb+2 issued before AV of tb)
    to hide exp latency and keep the PE HAM-warm (2.4 GHz).
  - softmax normalize: copy psum out early + reciprocal_approx_fast +
    gpsimd partition_broadcast (no DRAM round-trip, no psum stalls).
  - wout/bout loads deferred past the x loads (kills the startup stall).
"""

import contextlib
import ctypes
import sys
import types

import ml_dtypes
import numpy as np

import concourse.bacc as bacc
import concourse.mybir as mybir
import concourse.tile as tile
from concourse.bass import ds

N_CORES = 8
B = 2
S = 2048
D = 1024
HD = 64
N_HEADS = 16

BF = mybir.dt.bfloat16
F32 = mybir.dt.float32
F8 = mybir.dt.float8e4

SC = 512          # seq chunk (moving-operand width)
N_SC = S // SC    # 4
N_DC = D // 128   # 8 contraction chunks
N_SB = S // 128   # 16 seq 128-blocks

NPBF = ml_dtypes.bfloat16


def _build_program(dbg=False):
    nc = bacc.Bacc(
        "TRN2", target_bir_lowering=False, debug=False, num_devices=N_CORES
    )

    # xt[b, sc, dc, p, col] = x[b, sc*512+col, 128*dc+p]
    xt_d = nc.dram_tensor("xt", [B, N_SC, N_DC, 128, SC], BF, kind="ExternalInput").ap()
    wq_d = nc.dram_tensor("wq", [128, N_DC, 128], BF, kind="ExternalInput").ap()
    wk_d = nc.dram_tensor("wk", [128, N_DC, 128], BF, kind="ExternalInput").ap()
    wv_d = nc.dram_tensor("wv", [128, N_DC, 128], BF, kind="ExternalInput").ap()
    bq_d = nc.dram_tensor("bq", [128, 1], F32, kind="ExternalInput").ap()
    bk_d = nc.dram_tensor("bk", [128, 1], F32, kind="ExternalInput").ap()
    bv4_d = nc.dram_tensor("bv4", [1, 512], F32, kind="ExternalInput").ap()
    tri_d = nc.dram_tensor("tri", [128, 128], BF, kind="ExternalInput").ap()
    wout_d = nc.dram_tensor("wout", [128, N_DC, D], BF, kind="ExternalInput").ap()
    bout_d = nc.dram_tensor("bout", [1, D], F32, kind="ExternalInput").ap()
    out_d = nc.dram_tensor("out", [512, D], F32, kind="ExternalOutput").ap()

    y_part = nc.dram_tensor("y_part", [8, 128, SC], BF)
    y_all = nc.dram_tensor("y_all", [8, 128, SC], BF)
    z_bounce = nc.dram_tensor("z_bounce", [4, 1, SC], BF)
    if dbg:
        dbg_qT = nc.dram_tensor("dbg_qT", [B, 64, 2, S], BF, kind="ExternalOutput").ap()
        dbg_kT = nc.dram_tensor("dbg_kT", [B, 64, 2, S], BF, kind="ExternalOutput").ap()
        dbg_v = nc.dram_tensor("dbg_v", [B, 128, N_SB * 2 * 65], BF, kind="ExternalOutput").ap()
        dbg_z = nc.dram_tensor("dbg_z", [B, 4, 2, 1, SC], mybir.dt.float32, kind="ExternalOutput").ap()
        dbg_ex = nc.dram_tensor("dbg_ex", [B, 4, 2, 128, SC], BF, kind="ExternalOutput").ap()
        dbg_yp = nc.dram_tensor("dbg_yp", [8, 128, SC], BF, kind="ExternalOutput").ap()

    with tile.TileContext(nc) as tc, contextlib.ExitStack() as ctx:
        const = ctx.enter_context(tc.tile_pool(name="const", bufs=1))
        xt_pool = ctx.enter_context(tc.tile_pool(name="xt", bufs=18))
        qk_pool = ctx.enter_context(tc.tile_pool(name="qk", bufs=2))
        v_pool = ctx.enter_context(tc.tile_pool(name="vp", bufs=2))
        exp_pool = ctx.enter_context(tc.tile_pool(name="expp", bufs=6))
        yn_pool = ctx.enter_context(tc.tile_pool(name="yn", bufs=3))
        z_pool = ctx.enter_context(tc.tile_pool(name="zp", bufs=2))
        yg_pool = ctx.enter_context(tc.tile_pool(name="yg", bufs=1))
        outs_pool = ctx.enter_context(tc.tile_pool(name="outs", bufs=3))
        psum = ctx.enter_context(tc.tile_pool(name="psum", bufs=1, space="PSUM"))

        # ---- early constants (needed by QKV/attention) ----
        # 2 contiguous-chunk loads per weight parallelize across DMA queues
        wq_sb = const.tile([128, N_DC, 128], BF)
        wk_sb = const.tile([128, N_DC, 128], BF)
        wv_sb = const.tile([128, N_DC, 128], BF)
        for w_sb, w_d in ((wq_sb, wq_d), (wk_sb, wk_d), (wv_sb, wv_d)):
            nc.sync.dma_start(out=w_sb[:, 0:4, :], in_=w_d[:, 0:4, :])
            nc.sync.dma_start(out=w_sb[:, 4:8, :], in_=w_d[:, 4:8, :])
        bq_sb = const.tile([128, 1], F32)
        nc.sync.dma_start(out=bq_sb, in_=bq_d)
        bk_sb = const.tile([128, 1], F32)
        nc.sync.dma_start(out=bk_sb, in_=bk_d)
        bv4_bc = const.tile([128, 512], F32)
        nc.sync.dma_start(out=bv4_bc, in_=bv4_d.to_broadcast([128, 512]))
        tri_sb = const.tile([128, 128], BF)
        nc.sync.dma_start(out=tri_sb, in_=tri_d)

        for b in range(B):
            # ---- QKV projection for batch b (2 heads) ----
            qT = qk_pool.tile([64, 2, S], BF, tag="qT")
            kT = qk_pool.tile([64, 2, S], BF, tag="kT")
            v_sb = v_pool.tile([128, N_SB, 2, 65], BF)
            nc.vector.memset(v_sb[:, :, :, 64:65], 1.0)
            for sc in range(N_SC):
                xts = []
                for dc in range(N_DC):
                    xt = xt_pool.tile([128, SC], BF)
                    nc.sync.dma_start(out=xt, in_=xt_d[b, sc, dc])
                    xts.append(xt)
                psq = psum.tile([128, SC], F32, tag="qkv", bufs=1)
                for dc in range(N_DC):
                    nc.tensor.matmul(
                        psq, wq_sb[:, dc, :], xts[dc],
                        start=(dc == 0), stop=(dc == N_DC - 1),
                    )
                for h in range(2):
                    nc.vector.tensor_scalar_add(
                        out=qT[:, h, ds(sc * SC, SC)],
                        in0=psq[ds(64 * h, 64), :],
                        scalar1=bq_sb[ds(64 * h, 64), :],
                    )

                psk = psum.tile([128, SC], F32, tag="qkv", bufs=1)
                for dc in range(N_DC):
                    nc.tensor.matmul(
                        psk, wk_sb[:, dc, :], xts[dc],
                        start=(dc == 0), stop=(dc == N_DC - 1),
                    )
                for h in range(2):
                    nc.vector.tensor_scalar_add(
                        out=kT[:, h, ds(sc * SC, SC)],
                        in0=psk[ds(64 * h, 64), :],
                        scalar1=bk_sb[ds(64 * h, 64), :],
                    )

                psv = psum.tile([128, 4, 128], F32, tag="psv", bufs=1)
                for j4 in range(4):
                    for dc in range(N_DC):
                        nc.tensor.matmul(
                            psv[:, j4, :],
                            xts[dc][:, ds(j4 * 128, 128)],
                            wv_sb[:, dc, :],
                            start=(dc == 0), stop=(dc == N_DC - 1),
                            skip_group_check=True,
                        )
                nc.vector.tensor_add(
                    out=v_sb[:, ds(4 * sc, 4), :, 0:64],
                    in0=psv.rearrange("p j (h e) -> p j h e", h=2),
                    in1=bv4_bc.rearrange("p (j h e) -> p j h e", j=4, h=2),
                )

            # ---- attention for batch b ----
            for qc in range(N_SC):
                ntb = 4 * qc + 4
                psys = [
                    psum.tile([65, SC], F32, tag="psy", bufs=2, name=f"psy{h}")
                    for h in range(2)
                ]
                exs = {}

                def issue_score(tb, qc=qc, exs=exs):
                    j = tb - 4 * qc  # >= 0: diagonal block index
                    off = 128 * j if j >= 0 else 0
                    w = SC - off
                    for h in range(2):
                        pss = psum.tile([128, SC], F32, tag="pss", bufs=4)
                        nc.tensor.matmul(
                            pss[:, off:],
                            kT[:, h, ds(tb * 128, 128)],
                            qT[:, h, ds(qc * SC + off, w)],
                            start=True, stop=True,
                        )
                        ex = exp_pool.tile([128, SC], BF)
                        nc.scalar.activation(
                            out=ex[:, off:], in_=pss[:, off:],
                            func=mybir.ActivationFunctionType.Exp,
                            scale=0.125,
                        )
                        if j >= 0:
                            nc.vector.tensor_mul(
                                out=ex[:, ds(off, 128)],
                                in0=ex[:, ds(off, 128)],
                                in1=tri_sb,
                            )
                        if dbg and tb == 0:
                            nc.sync.dma_start(out=dbg_ex[b, qc, h], in_=ex)
                        exs[(tb, h)] = (ex, off)

                issue_score(0)
                if ntb > 1:
                    issue_score(1)
                for tb in range(ntb):
                    if tb + 2 < ntb:
                        issue_score(tb + 2)
                    for h in range(2):
                        ex, off = exs.pop((tb, h))
                        nc.tensor.matmul(
                            psys[h][:, off:],
                            v_sb[:, tb, h, :],
                            ex[:, off:],
                            start=(tb == 0), stop=(tb == ntb - 1),
                            skip_group_check=True,
                        )
                for h in range(2):
                    psy = psys[h]
                    yraw = yn_pool.tile([64, SC], BF, tag=f"yraw{h}")
                    nc.vector.tensor_copy(out=yraw, in_=psy[0:64, :])
                    zs = z_pool.tile([1, SC], F32, tag=f"zs{h}")
                    nc.vector.tensor_copy(out=zs, in_=psy[64:65, :])
                    zr = z_pool.tile([1, SC], F32, tag=f"zr{h}")
                    nc.vector.reciprocal_approx_fast(out=zr, in_=zs)
                    if dbg:
                        nc.sync.dma_start(out=dbg_z[b, qc, h], in_=zr)
                    zb = z_pool.tile([1, SC], BF, tag=f"zb{h}")
                    nc.vector.tensor_copy(out=zb, in_=zr)
                    zd = z_bounce.ap()[(2 * qc + h) % 4]
                    nc.sync.dma_start(out=zd, in_=zb)
                    zbb = z_pool.tile([64, SC], BF, tag=f"zbb{h}")
                    nc.sync.dma_start(out=zbb, in_=zd.to_broadcast([64, SC]))
                    yts = yn_pool.tile([64, SC], BF, tag=f"yts{h}")
                    nc.vector.tensor_mul(out=yts, in0=yraw, in1=zbb)
                    nc.sync.dma_start(
                        out=y_part.ap()[b * 4 + qc, ds(64 * h, 64), :], in_=yts
                    )

            if dbg:
                nc.sync.dma_start(out=dbg_qT[b], in_=qT)
                nc.sync.dma_start(out=dbg_kT[b], in_=kT)
                nc.sync.dma_start(
                    out=dbg_v[b], in_=v_sb.rearrange("p a b c -> p (a b c)")
                )

        if dbg:
            nc.sync.dma_start(out=dbg_yp, in_=y_part.ap())

        # ---- late constants (output projection) ----
        wout_sb = const.tile([128, N_DC, D], BF)
        nc.sync.dma_start(out=wout_sb, in_=wout_d)
        bout_bc = const.tile([128, D], F32)
        nc.sync.dma_start(out=bout_bc, in_=bout_d.to_broadcast([128, D]))

        # ---- reshard: head-split -> row-split ----
        nc.gpsimd.collective_compute(
            "AllToAll",
            mybir.AluOpType.bypass,
            replica_groups=[list(range(N_CORES))],
            ins=[y_part.ap()],
            outs=[y_all.ap()],
        )

        # keep the PE HAM-warm while waiting on the collective: dependency-free
        # filler matmuls into a never-read psum tile
        for f in range(24):
            psf = psum.tile([128, SC], F32, tag="pss", bufs=4)
            nc.tensor.matmul(
                psf, wq_sb[:, f % 8, :], wout_sb[:, f % 8, ds(0, SC)],
                start=True, stop=True,
            )

        # ---- output projection for this core's 512 rows ----
        ygs = []
        for ec in range(8):
            yg = yg_pool.tile([128, SC], BF, tag=f"yg{ec}")
            nc.sync.dma_start(out=yg, in_=y_all.ap()[ec])
            ygs.append(yg)
        for sb in range(4):
            for ch in range(2):
                pso = psum.tile([128, SC], F32, tag="pss", bufs=4)
                for ec in range(8):
                    nc.tensor.matmul(
                        pso,
                        ygs[ec][:, ds(sb * 128, 128)],
                        wout_sb[:, ec, ds(ch * SC, SC)],
                        start=(ec == 0), stop=(ec == 7),
                    )
                ot = outs_pool.tile([128, SC], F32)
                nc.vector.tensor_add(
                    out=ot, in0=pso, in1=bout_bc[:, ds(ch * SC, SC)]
                )
                nc.sync.dma_start(
                    out=out_d[ds(sb * 128, 128), ds(ch * SC, SC)], in_=ot
                )

    nc.compile()
    return nc


_NC_CACHE = None


def _get_program():
    global _NC_CACHE
    if _NC_CACHE is None:
        _NC_CACHE = _build_program()
    return _NC_CACHE


def make_in_maps(x, Wqkv, bqkv, Wout, bout):
    x = np.asarray(x, dtype=np.float32)
    Wqkv = np.asarray(Wqkv, dtype=np.float32)
    bqkv = np.asarray(bqkv, dtype=np.float32)
    Wout = np.asarray(Wout, dtype=np.float32)
    bout = np.asarray(bout, dtype=np.float32)

    # xt[b, sc, dc, p, col] = x[b, sc*512+col, 128*dc+p]
    xt = np.ascontiguousarray(
        x.reshape(B, N_SC, SC, N_DC, 128).transpose(0, 1, 3, 4, 2)
    ).astype(NPBF)
    wout_t = np.ascontiguousarray(
        Wout.reshape(N_DC, 128, D).transpose(1, 0, 2)
    ).astype(NPBF)
    bout2 = np.ascontiguousarray(bout.reshape(1, D))
    tri = np.triu(np.ones((128, 128), dtype=np.float32)).astype(NPBF)

    def wslice(lo, hi, c):
        h0, h1 = 2 * c, 2 * c + 1
        w = np.concatenate([Wqkv[h0, :, lo:hi], Wqkv[h1, :, lo:hi]], axis=1)
        return np.ascontiguousarray(
            w.reshape(N_DC, 128, 128).transpose(1, 0, 2)
        ).astype(NPBF)

    in_maps = []
    for c in range(N_CORES):
        h0, h1 = 2 * c, 2 * c + 1
        bq = np.concatenate([bqkv[h0, 0:64], bqkv[h1, 0:64]]).reshape(128, 1)
        bk = np.concatenate([bqkv[h0, 64:128], bqkv[h1, 64:128]]).reshape(128, 1)
        bvp = np.concatenate([bqkv[h0, 128:192], bqkv[h1, 128:192]])
        bv4 = np.tile(bvp, 4).reshape(1, 512)
        in_maps.append(
            {
                "xt": xt,
                "wq": wslice(0, 64, c),
                "wk": wslice(64, 128, c),
                "wv": wslice(128, 192, c),
                "bq": np.ascontiguousarray(bq),
                "bk": np.ascontiguousarray(bk),
                "bv4": np.ascontiguousarray(bv4),
                "tri": tri,
                "wout": wout_t,
                "bout": bout2,
            }
        )
    return in_maps


def assemble(results):
    full = np.empty((N_CORES * 512, D), dtype=np.float32)
    for c in range(N_CORES):
        full[512 * c : 512 * (c + 1)] = results[c]["out"]
    return full.reshape(B, S, D)


def _install_ntff_hook():
    """The agent image's antenv lacks axon_hooks; provide it so
    run_bass_kernel_spmd(trace=True) can NTFF-profile via libaxon."""
    if "antenv.axon_hooks" in sys.modules:
        return
    so_path = "/opt/axon/libaxon_pjrt.so"
    try:
        lib = ctypes.CDLL(so_path)
        lib.axon_start_nrt_profile.argtypes = [
            ctypes.POINTER(ctypes.c_int64),
            ctypes.c_size_t,
        ]
        lib.axon_start_nrt_profile.restype = ctypes.c_int64
        lib.axon_stop_nrt_profile.argtypes = [ctypes.c_char_p]
        lib.axon_stop_nrt_profile.restype = ctypes.c_int64
    except (OSError, AttributeError):
        return

    @contextlib.contextmanager
    def _hook(output_dir, device_ids):
        import jax

        jax.devices()
        if device_ids:
            ids = (ctypes.c_int64 * len(device_ids))(*device_ids)
            rc = lib.axon_start_nrt_profile(ids, len(device_ids))
        else:
            rc = lib.axon_start_nrt_profile(None, 0)
        if rc != 0:
            raise RuntimeError(f"axon_start_nrt_profile rc={rc}")
        try:
            yield
        finally:
            n = lib.axon_stop_nrt_profile(str(output_dir).encode())
            if n < 0:
                raise RuntimeError(f"axon_stop_nrt_profile rc={n}")

    mod = types.ModuleType("antenv.axon_hooks")
    mod.get_axon_ntff_profile_hook = lambda: _hook
    mod.set_axon_ntff_profile_hook = lambda h: None
    sys.modules["antenv.axon_hooks"] = mod


def run(inputs, trace=False):
    """Run on the 8 NeuronCores. Returns (output, BassKernelResults)."""
    from concourse.bass_utils import run_bass_kernel_spmd

    if trace:
        _install_ntff_hook()
    nc = _get_program()
    in_maps = make_in_maps(**inputs)
    res = run_bass_kernel_spmd(
        nc, in_maps, core_ids=list(range(N_CORES)), trace=trace
    )
    return assemble(res.results), res


def kernel(x, Wqkv, bqkv, Wout, bout):
    out, _ = run(
        {"x": x, "Wqkv": Wqkv, "bqkv": bqkv, "Wout": Wout, "bout": bout},
        trace=False,
    )
    return out


# revision 32
# speedup vs baseline: 1.1890x; 1.0302x over previous
"""Trainium2 Bass kernel for naive causal MHA (dense transformer block).

Problem: x[2, 2048, 1024], per-head QKV (16 heads, head_dim 64), causal
softmax attention, concat heads, output projection.

Sharding (8 NeuronCores, tensor-parallel over heads):
  - core c computes QKV + attention for heads {2c, 2c+1} over both batches,
    in a transposed layout: scores are built as [keys, queries] so the
    softmax denominator comes from an extra ones-column in V and the
    attention output lands directly in the [head_dim, seq] layout the
    output projection needs as its stationary operand.
  - an 8-way AllToAll (bf16) reshards y from head-split to row-split,
  - each core computes a disjoint 512-row slice of y @ Wout + bout.

v2 vs baseline:
  - bf16 storage/matmul operands everywhere (fp32 PSUM accumulate);
    halves DMA + collective bytes, enables fast weight loads.
  - causal trimming: diagonal score blocks only compute/exp the valid
    column range; AV matmuls stream the valid subrange.
  - 2-deep software pipeline (scores for tb+2 issued before AV of tb)
    to hide exp latency and keep the PE HAM-warm (2.4 GHz).
  - softmax normalize: copy psum out early + reciprocal_approx_fast +
    gpsimd partition_broadcast (no DRAM round-trip, no psum stalls).
  - wout/bout loads deferred past the x loads (kills the startup stall).
"""

import contextlib
import ctypes
import sys
import types

import ml_dtypes
import numpy as np

import concourse.bacc as bacc
import concourse.mybir as mybir
import concourse.tile as tile
from concourse.bass import ds

N_CORES = 8
B = 2
S = 2048
D = 1024
HD = 64
N_HEADS = 16

BF = mybir.dt.bfloat16
F32 = mybir.dt.float32
F8 = mybir.dt.float8e4

SC = 512          # seq chunk (moving-operand width)
N_SC = S // SC    # 4
N_DC = D // 128   # 8 contraction chunks
N_SB = S // 128   # 16 seq 128-blocks

NPBF = ml_dtypes.bfloat16


def _build_program(dbg=False):
    nc = bacc.Bacc(
        "TRN2", target_bir_lowering=False, debug=False, num_devices=N_CORES
    )

    # xt[b, sc, dc, p, col] = x[b, sc*512+col, 128*dc+p]
    xt_d = nc.dram_tensor("xt", [B, N_SC, N_DC, 128, SC], BF, kind="ExternalInput").ap()
    wq_d = nc.dram_tensor("wq", [128, N_DC, 128], BF, kind="ExternalInput").ap()
    wk_d = nc.dram_tensor("wk", [128, N_DC, 128], BF, kind="ExternalInput").ap()
    wv_d = nc.dram_tensor("wv", [128, N_DC, 128], BF, kind="ExternalInput").ap()
    bq_d = nc.dram_tensor("bq", [128, 1], F32, kind="ExternalInput").ap()
    bk_d = nc.dram_tensor("bk", [128, 1], F32, kind="ExternalInput").ap()
    bv4_d = nc.dram_tensor("bv4", [1, 512], F32, kind="ExternalInput").ap()
    tri_d = nc.dram_tensor("tri", [128, 128], BF, kind="ExternalInput").ap()
    wout_d = nc.dram_tensor("wout", [128, N_DC, D], BF, kind="ExternalInput").ap()
    bout_d = nc.dram_tensor("bout", [1, D], F32, kind="ExternalInput").ap()
    out_d = nc.dram_tensor("out", [512, D], F32, kind="ExternalOutput").ap()

    y_part = nc.dram_tensor("y_part", [8, 128, SC], BF)
    y_all = nc.dram_tensor("y_all", [8, 128, SC], BF)
    z_bounce = nc.dram_tensor("z_bounce", [4, 1, SC], BF)
    if dbg:
        dbg_qT = nc.dram_tensor("dbg_qT", [B, 64, 2, S], BF, kind="ExternalOutput").ap()
        dbg_kT = nc.dram_tensor("dbg_kT", [B, 64, 2, S], BF, kind="ExternalOutput").ap()
        dbg_v = nc.dram_tensor("dbg_v", [B, 128, N_SB * 2 * 65], BF, kind="ExternalOutput").ap()
        dbg_z = nc.dram_tensor("dbg_z", [B, 4, 2, 1, SC], mybir.dt.float32, kind="ExternalOutput").ap()
        dbg_ex = nc.dram_tensor("dbg_ex", [B, 4, 2, 128, SC], BF, kind="ExternalOutput").ap()
        dbg_yp = nc.dram_tensor("dbg_yp", [8, 128, SC], BF, kind="ExternalOutput").ap()

    with tile.TileContext(nc) as tc, contextlib.ExitStack() as ctx:
        const = ctx.enter_context(tc.tile_pool(name="const", bufs=1))
        xt_pool = ctx.enter_context(tc.tile_pool(name="xt", bufs=18))
        qk_pool = ctx.enter_context(tc.tile_pool(name="qk", bufs=2))
        v_pool = ctx.enter_context(tc.tile_pool(name="vp", bufs=2))
        exp_pool = ctx.enter_context(tc.tile_pool(name="expp", bufs=6))
        yn_pool = ctx.enter_context(tc.tile_pool(name="yn", bufs=3))
        z_pool = ctx.enter_context(tc.tile_pool(name="zp", bufs=2))
        yg_pool = ctx.enter_context(tc.tile_pool(name="yg", bufs=1))
        outs_pool = ctx.enter_context(tc.tile_pool(name="outs", bufs=3))
        psum = ctx.enter_context(tc.tile_pool(name="psum", bufs=1, space="PSUM"))

        # ---- early constants (needed by QKV/attention) ----
        wq_sb = const.tile([128, N_DC, 128], BF)
        nc.sync.dma_start(out=wq_sb, in_=wq_d)
        wk_sb = const.tile([128, N_DC, 128], BF)
        nc.sync.dma_start(out=wk_sb, in_=wk_d)
        wv_sb = const.tile([128, N_DC, 128], BF)
        nc.sync.dma_start(out=wv_sb, in_=wv_d)
        bq_sb = const.tile([128, 1], F32)
        nc.sync.dma_start(out=bq_sb, in_=bq_d)
        bk_sb = const.tile([128, 1], F32)
        nc.sync.dma_start(out=bk_sb, in_=bk_d)
        bv4_bc = const.tile([128, 512], F32)
        nc.sync.dma_start(out=bv4_bc, in_=bv4_d.to_broadcast([128, 512]))
        tri_sb = const.tile([128, 128], BF)
        nc.sync.dma_start(out=tri_sb, in_=tri_d)

        for b in range(B):
            # ---- QKV projection for batch b (2 heads) ----
            qT = qk_pool.tile([64, 2, S], BF, tag="qT")
            kT = qk_pool.tile([64, 2, S], BF, tag="kT")
            v_sb = v_pool.tile([128, N_SB, 2, 65], BF)
            nc.vector.memset(v_sb[:, :, :, 64:65], 1.0)
            for sc in range(N_SC):
                xts = []
                for dc in range(N_DC):
                    xt = xt_pool.tile([128, SC], BF)
                    nc.sync.dma_start(out=xt, in_=xt_d[b, sc, dc])
                    xts.append(xt)
                psq = psum.tile([128, SC], F32, tag="qkv", bufs=1)
                for dc in range(N_DC):
                    nc.tensor.matmul(
                        psq, wq_sb[:, dc, :], xts[dc],
                        start=(dc == 0), stop=(dc == N_DC - 1),
                    )
                for h in range(2):
                    nc.vector.tensor_scalar_add(
                        out=qT[:, h, ds(sc * SC, SC)],
                        in0=psq[ds(64 * h, 64), :],
                        scalar1=bq_sb[ds(64 * h, 64), :],
                    )

                psk = psum.tile([128, SC], F32, tag="qkv", bufs=1)
                for dc in range(N_DC):
                    nc.tensor.matmul(
                        psk, wk_sb[:, dc, :], xts[dc],
                        start=(dc == 0), stop=(dc == N_DC - 1),
                    )
                for h in range(2):
                    nc.vector.tensor_scalar_add(
                        out=kT[:, h, ds(sc * SC, SC)],
                        in0=psk[ds(64 * h, 64), :],
                        scalar1=bk_sb[ds(64 * h, 64), :],
                    )

                psv = psum.tile([128, 4, 128], F32, tag="psv", bufs=1)
                for j4 in range(4):
                    for dc in range(N_DC):
                        nc.tensor.matmul(
                            psv[:, j4, :],
                            xts[dc][:, ds(j4 * 128, 128)],
                            wv_sb[:, dc, :],
                            start=(dc == 0), stop=(dc == N_DC - 1),
                            skip_group_check=True,
                        )
                nc.vector.tensor_add(
                    out=v_sb[:, ds(4 * sc, 4), :, 0:64],
                    in0=psv.rearrange("p j (h e) -> p j h e", h=2),
                    in1=bv4_bc.rearrange("p (j h e) -> p j h e", j=4, h=2),
                )

            # ---- attention for batch b ----
            for qc in range(N_SC):
                ntb = 4 * qc + 4
                psys = [
                    psum.tile([65, SC], F32, tag="psy", bufs=2, name=f"psy{h}")
                    for h in range(2)
                ]
                exs = {}

                def issue_score(tb, qc=qc, exs=exs):
                    j = tb - 4 * qc  # >= 0: diagonal block index
                    off = 128 * j if j >= 0 else 0
                    w = SC - off
                    for h in range(2):
                        pss = psum.tile([128, SC], F32, tag="pss", bufs=4)
                        nc.tensor.matmul(
                            pss[:, off:],
                            kT[:, h, ds(tb * 128, 128)],
                            qT[:, h, ds(qc * SC + off, w)],
                            start=True, stop=True,
                        )
                        ex = exp_pool.tile([128, SC], BF)
                        nc.scalar.activation(
                            out=ex[:, off:], in_=pss[:, off:],
                            func=mybir.ActivationFunctionType.Exp,
                            scale=0.125,
                        )
                        if j >= 0:
                            nc.vector.tensor_mul(
                                out=ex[:, ds(off, 128)],
                                in0=ex[:, ds(off, 128)],
                                in1=tri_sb,
                            )
                        if dbg and tb == 0:
                            nc.sync.dma_start(out=dbg_ex[b, qc, h], in_=ex)
                        exs[(tb, h)] = (ex, off)

                issue_score(0)
                if ntb > 1:
                    issue_score(1)
                for tb in range(ntb):
                    if tb + 2 < ntb:
                        issue_score(tb + 2)
                    for h in range(2):
                        ex, off = exs.pop((tb, h))
                        nc.tensor.matmul(
                            psys[h][:, off:],
                            v_sb[:, tb, h, :],
                            ex[:, off:],
                            start=(tb == 0), stop=(tb == ntb - 1),
                            skip_group_check=True,
                        )
                for h in range(2):
                    psy = psys[h]
                    yraw = yn_pool.tile([64, SC], BF, tag=f"yraw{h}")
                    nc.vector.tensor_copy(out=yraw, in_=psy[0:64, :])
                    zs = z_pool.tile([1, SC], F32, tag=f"zs{h}")
                    nc.vector.tensor_copy(out=zs, in_=psy[64:65, :])
                    zr = z_pool.tile([1, SC], F32, tag=f"zr{h}")
                    nc.vector.reciprocal_approx_fast(out=zr, in_=zs)
                    if dbg:
                        nc.sync.dma_start(out=dbg_z[b, qc, h], in_=zr)
                    zb = z_pool.tile([1, SC], BF, tag=f"zb{h}")
                    nc.vector.tensor_copy(out=zb, in_=zr)
                    zd = z_bounce.ap()[(2 * qc + h) % 4]
                    nc.sync.dma_start(out=zd, in_=zb)
                    zbb = z_pool.tile([64, SC], BF, tag=f"zbb{h}")
                    nc.sync.dma_start(out=zbb, in_=zd.to_broadcast([64, SC]))
                    yts = yn_pool.tile([64, SC], BF, tag=f"yts{h}")
                    nc.vector.tensor_mul(out=yts, in0=yraw, in1=zbb)
                    nc.sync.dma_start(
                        out=y_part.ap()[b * 4 + qc, ds(64 * h, 64), :], in_=yts
                    )

            if dbg:
                nc.sync.dma_start(out=dbg_qT[b], in_=qT)
                nc.sync.dma_start(out=dbg_kT[b], in_=kT)
                nc.sync.dma_start(
                    out=dbg_v[b], in_=v_sb.rearrange("p a b c -> p (a b c)")
                )

        if dbg:
            nc.sync.dma_start(out=dbg_yp, in_=y_part.ap())

        # ---- late constants (output projection) ----
        wout_sb = const.tile([128, N_DC, D], BF)
        nc.sync.dma_start(out=wout_sb, in_=wout_d)
        bout_bc = const.tile([128, D], F32)
        nc.sync.dma_start(out=bout_bc, in_=bout_d.to_broadcast([128, D]))

        # keep the PE HAM-warm through the normalize drain + collective
        # rendezvous: dependency-free filler matmuls into a never-read psum
        # tile. The next real PE work (out-proj) waits on the collective
        # anyway, so these are free.
        for f in range(96):
            psf = psum.tile([128, SC], F32, tag="pss", bufs=4)
            nc.tensor.matmul(
                psf, wq_sb[:, f % 8, :], wq_sb[:, 0:4, :],
                start=True, stop=True,
            )

        # ---- reshard: head-split -> row-split ----
        nc.gpsimd.collective_compute(
            "AllToAll",
            mybir.AluOpType.bypass,
            replica_groups=[list(range(N_CORES))],
            ins=[y_part.ap()],
            outs=[y_all.ap()],
        )

        # ---- output projection for this core's 512 rows ----
        ygs = []
        for ec in range(8):
            yg = yg_pool.tile([128, SC], BF, tag=f"yg{ec}")
            nc.sync.dma_start(out=yg, in_=y_all.ap()[ec])
            ygs.append(yg)
        for sb in range(4):
            for ch in range(2):
                pso = psum.tile([128, SC], F32, tag="pss", bufs=4)
                for ec in range(8):
                    nc.tensor.matmul(
                        pso,
                        ygs[ec][:, ds(sb * 128, 128)],
                        wout_sb[:, ec, ds(ch * SC, SC)],
                        start=(ec == 0), stop=(ec == 7),
                    )
                ot = outs_pool.tile([128, SC], F32)
                nc.vector.tensor_add(
                    out=ot, in0=pso, in1=bout_bc[:, ds(ch * SC, SC)]
                )
                nc.sync.dma_start(
                    out=out_d[ds(sb * 128, 128), ds(ch * SC, SC)], in_=ot
                )

    nc.compile()
    return nc


_NC_CACHE = None


def _get_program():
    global _NC_CACHE
    if _NC_CACHE is None:
        _NC_CACHE = _build_program()
    return _NC_CACHE


def make_in_maps(x, Wqkv, bqkv, Wout, bout):
    x = np.asarray(x, dtype=np.float32)
    Wqkv = np.asarray(Wqkv, dtype=np.float32)
    bqkv = np.asarray(bqkv, dtype=np.float32)
    Wout = np.asarray(Wout, dtype=np.float32)
    bout = np.asarray(bout, dtype=np.float32)

    # xt[b, sc, dc, p, col] = x[b, sc*512+col, 128*dc+p]
    xt = np.ascontiguousarray(
        x.reshape(B, N_SC, SC, N_DC, 128).transpose(0, 1, 3, 4, 2)
    ).astype(NPBF)
    wout_t = np.ascontiguousarray(
        Wout.reshape(N_DC, 128, D).transpose(1, 0, 2)
    ).astype(NPBF)
    bout2 = np.ascontiguousarray(bout.reshape(1, D))
    tri = np.triu(np.ones((128, 128), dtype=np.float32)).astype(NPBF)

    def wslice(lo, hi, c):
        h0, h1 = 2 * c, 2 * c + 1
        w = np.concatenate([Wqkv[h0, :, lo:hi], Wqkv[h1, :, lo:hi]], axis=1)
        return np.ascontiguousarray(
            w.reshape(N_DC, 128, 128).transpose(1, 0, 2)
        ).astype(NPBF)

    in_maps = []
    for c in range(N_CORES):
        h0, h1 = 2 * c, 2 * c + 1
        bq = np.concatenate([bqkv[h0, 0:64], bqkv[h1, 0:64]]).reshape(128, 1)
        bk = np.concatenate([bqkv[h0, 64:128], bqkv[h1, 64:128]]).reshape(128, 1)
        bvp = np.concatenate([bqkv[h0, 128:192], bqkv[h1, 128:192]])
        bv4 = np.tile(bvp, 4).reshape(1, 512)
        in_maps.append(
            {
                "xt": xt,
                "wq": wslice(0, 64, c),
                "wk": wslice(64, 128, c),
                "wv": wslice(128, 192, c),
                "bq": np.ascontiguousarray(bq),
                "bk": np.ascontiguousarray(bk),
                "bv4": np.ascontiguousarray(bv4),
                "tri": tri,
                "wout": wout_t,
                "bout": bout2,
            }
        )
    return in_maps


def assemble(results):
    full = np.empty((N_CORES * 512, D), dtype=np.float32)
    for c in range(N_CORES):
        full[512 * c : 512 * (c + 1)] = results[c]["out"]
    return full.reshape(B, S, D)


def _install_ntff_hook():
    """The agent image's antenv lacks axon_hooks; provide it so
    run_bass_kernel_spmd(trace=True) can NTFF-profile via libaxon."""
    if "antenv.axon_hooks" in sys.modules:
        return
    so_path = "/opt/axon/libaxon_pjrt.so"
    try:
        lib = ctypes.CDLL(so_path)
        lib.axon_start_nrt_profile.argtypes = [
            ctypes.POINTER(ctypes.c_int64),
            ctypes.c_size_t,
        ]
        lib.axon_start_nrt_profile.restype = ctypes.c_int64
        lib.axon_stop_nrt_profile.argtypes = [ctypes.c_char_p]
        lib.axon_stop_nrt_profile.restype = ctypes.c_int64
    except (OSError, AttributeError):
        return

    @contextlib.contextmanager
    def _hook(output_dir, device_ids):
        import jax

        jax.devices()
        if device_ids:
            ids = (ctypes.c_int64 * len(device_ids))(*device_ids)
            rc = lib.axon_start_nrt_profile(ids, len(device_ids))
        else:
            rc = lib.axon_start_nrt_profile(None, 0)
        if rc != 0:
            raise RuntimeError(f"axon_start_nrt_profile rc={rc}")
        try:
            yield
        finally:
            n = lib.axon_stop_nrt_profile(str(output_dir).encode())
            if n < 0:
                raise RuntimeError(f"axon_stop_nrt_profile rc={n}")

    mod = types.ModuleType("antenv.axon_hooks")
    mod.get_axon_ntff_profile_hook = lambda: _hook
    mod.set_axon_ntff_profile_hook = lambda h: None
    sys.modules["antenv.axon_hooks"] = mod


def run(inputs, trace=False):
    """Run on the 8 NeuronCores. Returns (output, BassKernelResults)."""
    from concourse.bass_utils import run_bass_kernel_spmd

    if trace:
        _install_ntff_hook()
    nc = _get_program()
    in_maps = make_in_maps(**inputs)
    res = run_bass_kernel_spmd(
        nc, in_maps, core_ids=list(range(N_CORES)), trace=trace
    )
    return assemble(res.results), res


def kernel(x, Wqkv, bqkv, Wout, bout):
    out, _ = run(
        {"x": x, "Wqkv": Wqkv, "bqkv": bqkv, "Wout": Wout, "bout": bout},
        trace=False,
    )
    return out


# revision 40
# speedup vs baseline: 1.2897x; 1.0847x over previous
"""Trainium2 Bass kernel for naive causal MHA (dense transformer block).

Problem: x[2, 2048, 1024], per-head QKV (16 heads, head_dim 64), causal
softmax attention, concat heads, output projection.

Sharding (8 NeuronCores, tensor-parallel over heads):
  - core c computes QKV + attention for heads {2c, 2c+1} over both batches,
    in a transposed layout: scores are built as [keys, queries] so the
    softmax denominator comes from an extra ones-column in V and the
    attention output lands directly in the [head_dim, seq] layout the
    output projection needs as its stationary operand.
  - an 8-way AllToAll (bf16) reshards y from head-split to row-split,
  - each core computes a disjoint 512-row slice of y @ Wout + bout.

v2 vs baseline:
  - bf16 storage/matmul operands everywhere (fp32 PSUM accumulate);
    halves DMA + collective bytes, enables fast weight loads.
  - causal trimming: diagonal score blocks only compute/exp the valid
    column range; AV matmuls stream the valid subrange.
  - 2-deep software pipeline (scores for tb+2 issued before AV of tb)
    to hide exp latency and keep the PE HAM-warm (2.4 GHz).
  - softmax normalize: copy psum out early + reciprocal_approx_fast +
    gpsimd partition_broadcast (no DRAM round-trip, no psum stalls).
  - wout/bout loads deferred past the x loads (kills the startup stall).
"""

import contextlib
import ctypes
import sys
import types

import ml_dtypes
import numpy as np

import concourse.bacc as bacc
import concourse.mybir as mybir
import concourse.tile as tile
from concourse.bass import ds

N_CORES = 8
B = 2
S = 2048
D = 1024
HD = 64
N_HEADS = 16

BF = mybir.dt.bfloat16
F32 = mybir.dt.float32
F8 = mybir.dt.float8e4

SC = 512          # seq chunk (moving-operand width)
N_SC = S // SC    # 4
N_DC = D // 128   # 8 contraction chunks
N_SB = S // 128   # 16 seq 128-blocks

NPBF = ml_dtypes.bfloat16


def _build_program(dbg=False):
    nc = bacc.Bacc(
        "TRN2", target_bir_lowering=False, debug=False, num_devices=N_CORES
    )

    # xt[b, sc, dc, p, col] = x[b, sc*512+col, 128*dc+p]
    xt_d = nc.dram_tensor("xt", [B, N_SC, N_DC, 128, SC], BF, kind="ExternalInput").ap()
    wq_d = nc.dram_tensor("wq", [128, N_DC, 128], BF, kind="ExternalInput").ap()
    wk_d = nc.dram_tensor("wk", [128, N_DC, 128], BF, kind="ExternalInput").ap()
    wv_d = nc.dram_tensor("wv", [128, N_DC, 128], BF, kind="ExternalInput").ap()
    bq_d = nc.dram_tensor("bq", [128, 1], F32, kind="ExternalInput").ap()
    bk_d = nc.dram_tensor("bk", [128, 1], F32, kind="ExternalInput").ap()
    bv4_d = nc.dram_tensor("bv4", [1, 512], F32, kind="ExternalInput").ap()
    tri_d = nc.dram_tensor("tri", [128, 128], BF, kind="ExternalInput").ap()
    wout_d = nc.dram_tensor("wout", [128, N_DC, D], BF, kind="ExternalInput").ap()
    bout_d = nc.dram_tensor("bout", [1, D], F32, kind="ExternalInput").ap()
    out_d = nc.dram_tensor("out", [512, D], F32, kind="ExternalOutput").ap()

    y_part = nc.dram_tensor("y_part", [8, 128, SC], BF)
    y_all = nc.dram_tensor("y_all", [8, 128, SC], BF)
    z_bounce = nc.dram_tensor("z_bounce", [4, 1, SC], BF)
    if dbg:
        dbg_qT = nc.dram_tensor("dbg_qT", [B, 64, 2, S], BF, kind="ExternalOutput").ap()
        dbg_kT = nc.dram_tensor("dbg_kT", [B, 64, 2, S], BF, kind="ExternalOutput").ap()
        dbg_v = nc.dram_tensor("dbg_v", [B, 128, N_SB * 2 * 128], BF, kind="ExternalOutput").ap()
        dbg_z = nc.dram_tensor("dbg_z", [B, 4, 2, 1, SC], mybir.dt.float32, kind="ExternalOutput").ap()
        dbg_ex = nc.dram_tensor("dbg_ex", [B, 4, 2, 128, SC], BF, kind="ExternalOutput").ap()
        dbg_yp = nc.dram_tensor("dbg_yp", [8, 128, SC], BF, kind="ExternalOutput").ap()

    with tile.TileContext(nc) as tc, contextlib.ExitStack() as ctx:
        const = ctx.enter_context(tc.tile_pool(name="const", bufs=1))
        xt_pool = ctx.enter_context(tc.tile_pool(name="xt", bufs=18))
        qk_pool = ctx.enter_context(tc.tile_pool(name="qk", bufs=2))
        v_pool = ctx.enter_context(tc.tile_pool(name="vp", bufs=2))
        exp_pool = ctx.enter_context(tc.tile_pool(name="expp", bufs=6))
        yn_pool = ctx.enter_context(tc.tile_pool(name="yn", bufs=3))
        z_pool = ctx.enter_context(tc.tile_pool(name="zp", bufs=2))
        yg_pool = ctx.enter_context(tc.tile_pool(name="yg", bufs=1))
        outs_pool = ctx.enter_context(tc.tile_pool(name="outs", bufs=3))
        psum = ctx.enter_context(tc.tile_pool(name="psum", bufs=1, space="PSUM"))

        # ---- early constants (needed by QKV/attention) ----
        wq_sb = const.tile([128, N_DC, 128], BF)
        nc.sync.dma_start(out=wq_sb, in_=wq_d)
        wk_sb = const.tile([128, N_DC, 128], BF)
        nc.sync.dma_start(out=wk_sb, in_=wk_d)
        wv_sb = const.tile([128, N_DC, 128], BF)
        nc.sync.dma_start(out=wv_sb, in_=wv_d)
        bq_sb = const.tile([128, 1], F32)
        nc.sync.dma_start(out=bq_sb, in_=bq_d)
        bk_sb = const.tile([128, 1], F32)
        nc.sync.dma_start(out=bk_sb, in_=bk_d)
        bv4_bc = const.tile([128, 512], F32)
        nc.sync.dma_start(out=bv4_bc, in_=bv4_d.to_broadcast([128, 512]))
        tri_sb = const.tile([128, 128], BF)
        nc.sync.dma_start(out=tri_sb, in_=tri_d)

        # q/k/v tiles padded to full 128-partition / 128-column matmul shapes:
        # half-array matmuls (64-deep scores, 65-wide AV) keep the PE at the
        # slow p-state (~2x slower). The pad regions are zeroed once here and
        # never written again (bias-adds only touch the live regions), so the
        # zero-padding survives the per-batch reuse.
        qkT_bufs = []
        for bi in range(B):
            qTb = qk_pool.tile([128, 2, S], BF, tag=f"qT{bi}", bufs=1)
            kTb = qk_pool.tile([128, 2, S], BF, tag=f"kT{bi}", bufs=1)
            nc.gpsimd.memset(qTb[64:128, :, :], 0.0)
            nc.gpsimd.memset(kTb[64:128, :, :], 0.0)
            qkT_bufs.append((qTb, kTb))
        v_bufs = []
        for bi in range(B):
            v_sbb = v_pool.tile([128, N_SB, 2, 128], BF, tag=f"v{bi}", bufs=1)
            nc.gpsimd.memset(v_sbb[:, :, :, 65:128], 0.0)
            nc.gpsimd.memset(v_sbb[:, :, :, 64:65], 1.0)
            v_bufs.append(v_sbb)

        for b in range(B):
            # ---- QKV projection for batch b (2 heads) ----
            qT, kT = qkT_bufs[b]
            v_sb = v_bufs[b]
            for sc in range(N_SC):
                xts = []
                for dc in range(N_DC):
                    xt = xt_pool.tile([128, SC], BF)
                    nc.sync.dma_start(out=xt, in_=xt_d[b, sc, dc])
                    xts.append(xt)
                psq = psum.tile([128, SC], F32, tag="qkv", bufs=1)
                for dc in range(N_DC):
                    nc.tensor.matmul(
                        psq, wq_sb[:, dc, :], xts[dc],
                        start=(dc == 0), stop=(dc == N_DC - 1),
                    )
                for h in range(2):
                    nc.vector.tensor_scalar_add(
                        out=qT[0:64, h, ds(sc * SC, SC)],
                        in0=psq[ds(64 * h, 64), :],
                        scalar1=bq_sb[ds(64 * h, 64), :],
                    )

                psk = psum.tile([128, SC], F32, tag="qkv", bufs=1)
                for dc in range(N_DC):
                    nc.tensor.matmul(
                        psk, wk_sb[:, dc, :], xts[dc],
                        start=(dc == 0), stop=(dc == N_DC - 1),
                    )
                for h in range(2):
                    nc.vector.tensor_scalar_add(
                        out=kT[0:64, h, ds(sc * SC, SC)],
                        in0=psk[ds(64 * h, 64), :],
                        scalar1=bk_sb[ds(64 * h, 64), :],
                    )

                psv = psum.tile([128, 4, 128], F32, tag="psv", bufs=1)
                for j4 in range(4):
                    for dc in range(N_DC):
                        nc.tensor.matmul(
                            psv[:, j4, :],
                            xts[dc][:, ds(j4 * 128, 128)],
                            wv_sb[:, dc, :],
                            start=(dc == 0), stop=(dc == N_DC - 1),
                            skip_group_check=True,
                        )
                nc.vector.tensor_add(
                    out=v_sb[:, ds(4 * sc, 4), :, 0:64],
                    in0=psv.rearrange("p j (h e) -> p j h e", h=2),
                    in1=bv4_bc.rearrange("p (j h e) -> p j h e", j=4, h=2),
                )

            # ---- attention for batch b ----
            for qc in range(N_SC):
                ntb = 4 * qc + 4
                psys = [
                    psum.tile([128, SC], F32, tag="psy", bufs=2, name=f"psy{h}")
                    for h in range(2)
                ]
                exs = {}

                def issue_score(tb, qc=qc, exs=exs):
                    j = tb - 4 * qc  # >= 0: diagonal block index
                    off = 128 * j if j >= 0 else 0
                    w = SC - off
                    for h in range(2):
                        pss = psum.tile([128, SC], F32, tag="pss", bufs=4)
                        nc.tensor.matmul(
                            pss[:, off:],
                            kT[:, h, ds(tb * 128, 128)],
                            qT[:, h, ds(qc * SC + off, w)],
                            start=True, stop=True,
                        )
                        ex = exp_pool.tile([128, SC], BF)
                        nc.scalar.activation(
                            out=ex[:, off:], in_=pss[:, off:],
                            func=mybir.ActivationFunctionType.Exp,
                            scale=0.125,
                        )
                        if j >= 0:
                            nc.vector.tensor_mul(
                                out=ex[:, ds(off, 128)],
                                in0=ex[:, ds(off, 128)],
                                in1=tri_sb,
                            )
                        if dbg and tb == 0:
                            nc.sync.dma_start(out=dbg_ex[b, qc, h], in_=ex)
                        exs[(tb, h)] = (ex, off)

                issue_score(0)
                if ntb > 1:
                    issue_score(1)
                for tb in range(ntb):
                    if tb + 2 < ntb:
                        issue_score(tb + 2)
                    for h in range(2):
                        ex, off = exs.pop((tb, h))
                        nc.tensor.matmul(
                            psys[h][:, off:],
                            v_sb[:, tb, h, :],
                            ex[:, off:],
                            start=(tb == 0), stop=(tb == ntb - 1),
                            skip_group_check=True,
                        )
                for h in range(2):
                    psy = psys[h]
                    yraw = yn_pool.tile([64, SC], BF, tag=f"yraw{h}")
                    nc.vector.tensor_copy(out=yraw, in_=psy[0:64, :])
                    zs = z_pool.tile([1, SC], F32, tag=f"zs{h}")
                    nc.vector.tensor_copy(out=zs, in_=psy[64:65, :])
                    zr = z_pool.tile([1, SC], F32, tag=f"zr{h}")
                    nc.vector.reciprocal_approx_fast(out=zr, in_=zs)
                    if dbg:
                        nc.sync.dma_start(out=dbg_z[b, qc, h], in_=zr)
                    zb = z_pool.tile([1, SC], BF, tag=f"zb{h}")
                    nc.vector.tensor_copy(out=zb, in_=zr)
                    zd = z_bounce.ap()[(2 * qc + h) % 4]
                    nc.sync.dma_start(out=zd, in_=zb)
                    zbb = z_pool.tile([64, SC], BF, tag=f"zbb{h}")
                    nc.sync.dma_start(out=zbb, in_=zd.to_broadcast([64, SC]))
                    yts = yn_pool.tile([64, SC], BF, tag=f"yts{h}")
                    nc.vector.tensor_mul(out=yts, in0=yraw, in1=zbb)
                    nc.sync.dma_start(
                        out=y_part.ap()[b * 4 + qc, ds(64 * h, 64), :], in_=yts
                    )

            if dbg:
                nc.sync.dma_start(out=dbg_qT[b], in_=qT)
                nc.sync.dma_start(out=dbg_kT[b], in_=kT)
                nc.sync.dma_start(
                    out=dbg_v[b], in_=v_sb.rearrange("p a b c -> p (a b c)")
                )

        if dbg:
            nc.sync.dma_start(out=dbg_yp, in_=y_part.ap())

        # ---- late constants (output projection) ----
        wout_sb = const.tile([128, N_DC, D], BF)
        nc.sync.dma_start(out=wout_sb, in_=wout_d)
        bout_bc = const.tile([128, D], F32)
        nc.sync.dma_start(out=bout_bc, in_=bout_d.to_broadcast([128, D]))

        # ---- reshard: head-split -> row-split ----
        nc.gpsimd.collective_compute(
            "AllToAll",
            mybir.AluOpType.bypass,
            replica_groups=[list(range(N_CORES))],
            ins=[y_part.ap()],
            outs=[y_all.ap()],
        )

        # ---- output projection for this core's 512 rows ----
        ygs = []
        for ec in range(8):
            yg = yg_pool.tile([128, SC], BF, tag=f"yg{ec}")
            nc.sync.dma_start(out=yg, in_=y_all.ap()[ec])
            ygs.append(yg)
        for sb in range(4):
            for ch in range(2):
                pso = psum.tile([128, SC], F32, tag="pss", bufs=4)
                for ec in range(8):
                    nc.tensor.matmul(
                        pso,
                        ygs[ec][:, ds(sb * 128, 128)],
                        wout_sb[:, ec, ds(ch * SC, SC)],
                        start=(ec == 0), stop=(ec == 7),
                    )
                ot = outs_pool.tile([128, SC], F32)
                nc.vector.tensor_add(
                    out=ot, in0=pso, in1=bout_bc[:, ds(ch * SC, SC)]
                )
                nc.sync.dma_start(
                    out=out_d[ds(sb * 128, 128), ds(ch * SC, SC)], in_=ot
                )

    nc.compile()
    return nc


_NC_CACHE = None


def _get_program():
    global _NC_CACHE
    if _NC_CACHE is None:
        _NC_CACHE = _build_program()
    return _NC_CACHE


def make_in_maps(x, Wqkv, bqkv, Wout, bout):
    x = np.asarray(x, dtype=np.float32)
    Wqkv = np.asarray(Wqkv, dtype=np.float32)
    bqkv = np.asarray(bqkv, dtype=np.float32)
    Wout = np.asarray(Wout, dtype=np.float32)
    bout = np.asarray(bout, dtype=np.float32)

    # xt[b, sc, dc, p, col] = x[b, sc*512+col, 128*dc+p]
    xt = np.ascontiguousarray(
        x.reshape(B, N_SC, SC, N_DC, 128).transpose(0, 1, 3, 4, 2)
    ).astype(NPBF)
    wout_t = np.ascontiguousarray(
        Wout.reshape(N_DC, 128, D).transpose(1, 0, 2)
    ).astype(NPBF)
    bout2 = np.ascontiguousarray(bout.reshape(1, D))
    tri = np.triu(np.ones((128, 128), dtype=np.float32)).astype(NPBF)

    def wslice(lo, hi, c):
        h0, h1 = 2 * c, 2 * c + 1
        w = np.concatenate([Wqkv[h0, :, lo:hi], Wqkv[h1, :, lo:hi]], axis=1)
        return np.ascontiguousarray(
            w.reshape(N_DC, 128, 128).transpose(1, 0, 2)
        ).astype(NPBF)

    in_maps = []
    for c in range(N_CORES):
        h0, h1 = 2 * c, 2 * c + 1
        bq = np.concatenate([bqkv[h0, 0:64], bqkv[h1, 0:64]]).reshape(128, 1)
        bk = np.concatenate([bqkv[h0, 64:128], bqkv[h1, 64:128]]).reshape(128, 1)
        bvp = np.concatenate([bqkv[h0, 128:192], bqkv[h1, 128:192]])
        bv4 = np.tile(bvp, 4).reshape(1, 512)
        in_maps.append(
            {
                "xt": xt,
                "wq": wslice(0, 64, c),
                "wk": wslice(64, 128, c),
                "wv": wslice(128, 192, c),
                "bq": np.ascontiguousarray(bq),
                "bk": np.ascontiguousarray(bk),
                "bv4": np.ascontiguousarray(bv4),
                "tri": tri,
                "wout": wout_t,
                "bout": bout2,
            }
        )
    return in_maps


def assemble(results):
    full = np.empty((N_CORES * 512, D), dtype=np.float32)
    for c in range(N_CORES):
        full[512 * c : 512 * (c + 1)] = results[c]["out"]
    return full.reshape(B, S, D)


def _install_ntff_hook():
    """The agent image's antenv lacks axon_hooks; provide it so
    run_bass_kernel_spmd(trace=True) can NTFF-profile via libaxon."""
    if "antenv.axon_hooks" in sys.modules:
        return
    so_path = "/opt/axon/libaxon_pjrt.so"
    try:
        lib = ctypes.CDLL(so_path)
        lib.axon_start_nrt_profile.argtypes = [
            ctypes.POINTER(ctypes.c_int64),
            ctypes.c_size_t,
        ]
        lib.axon_start_nrt_profile.restype = ctypes.c_int64
        lib.axon_stop_nrt_profile.argtypes = [ctypes.c_char_p]
        lib.axon_stop_nrt_profile.restype = ctypes.c_int64
    except (OSError, AttributeError):
        return

    @contextlib.contextmanager
    def _hook(output_dir, device_ids):
        import jax

        jax.devices()
        if device_ids:
            ids = (ctypes.c_int64 * len(device_ids))(*device_ids)
            rc = lib.axon_start_nrt_profile(ids, len(device_ids))
        else:
            rc = lib.axon_start_nrt_profile(None, 0)
        if rc != 0:
            raise RuntimeError(f"axon_start_nrt_profile rc={rc}")
        try:
            yield
        finally:
            n = lib.axon_stop_nrt_profile(str(output_dir).encode())
            if n < 0:
                raise RuntimeError(f"axon_stop_nrt_profile rc={n}")

    mod = types.ModuleType("antenv.axon_hooks")
    mod.get_axon_ntff_profile_hook = lambda: _hook
    mod.set_axon_ntff_profile_hook = lambda h: None
    sys.modules["antenv.axon_hooks"] = mod


def run(inputs, trace=False):
    """Run on the 8 NeuronCores. Returns (output, BassKernelResults)."""
    from concourse.bass_utils import run_bass_kernel_spmd

    if trace:
        _install_ntff_hook()
    nc = _get_program()
    in_maps = make_in_maps(**inputs)
    res = run_bass_kernel_spmd(
        nc, in_maps, core_ids=list(range(N_CORES)), trace=trace
    )
    return assemble(res.results), res


def kernel(x, Wqkv, bqkv, Wout, bout):
    out, _ = run(
        {"x": x, "Wqkv": Wqkv, "bqkv": bqkv, "Wout": Wout, "bout": bout},
        trace=False,
    )
    return out


# revision 44
# speedup vs baseline: 1.4487x; 1.1233x over previous
"""Trainium2 Bass kernel for naive causal MHA (dense transformer block).

Problem: x[2, 2048, 1024], per-head QKV (16 heads, head_dim 64), causal
softmax attention, concat heads, output projection.

Sharding (8 NeuronCores, tensor-parallel over heads):
  - core c computes QKV + attention for heads {2c, 2c+1} over both batches,
    in a transposed layout: scores are built as [keys, queries] so the
    softmax denominator comes from an extra ones-column in V and the
    attention output lands directly in the [head_dim, seq] layout the
    output projection needs as its stationary operand.
  - an 8-way AllToAll (bf16) reshards y from head-split to row-split,
  - each core computes a disjoint 512-row slice of y @ Wout + bout.

v2 vs baseline:
  - bf16 storage/matmul operands everywhere (fp32 PSUM accumulate);
    halves DMA + collective bytes, enables fast weight loads.
  - causal trimming: diagonal score blocks only compute/exp the valid
    column range; AV matmuls stream the valid subrange.
  - 2-deep software pipeline (scores for tb+2 issued before AV of tb)
    to hide exp latency and keep the PE HAM-warm (2.4 GHz).
  - softmax normalize: copy psum out early + reciprocal_approx_fast +
    gpsimd partition_broadcast (no DRAM round-trip, no psum stalls).
  - wout/bout loads deferred past the x loads (kills the startup stall).
"""

import contextlib
import ctypes
import sys
import types

import ml_dtypes
import numpy as np

import concourse.bacc as bacc
import concourse.mybir as mybir
import concourse.tile as tile
from concourse.bass import ds

N_CORES = 8
B = 2
S = 2048
D = 1024
HD = 64
N_HEADS = 16

BF = mybir.dt.bfloat16
F32 = mybir.dt.float32
F8 = mybir.dt.float8e4

SC = 512          # seq chunk (moving-operand width)
N_SC = S // SC    # 4
N_DC = D // 128   # 8 contraction chunks
N_SB = S // 128   # 16 seq 128-blocks

NPBF = ml_dtypes.bfloat16


def _build_program(dbg=False):
    nc = bacc.Bacc(
        "TRN2", target_bir_lowering=False, debug=False, num_devices=N_CORES
    )

    # xt[b, sc, dc, p, col] = x[b, sc*512+col, 128*dc+p]
    xt_d = nc.dram_tensor("xt", [B, N_SC, N_DC, 128, SC], BF, kind="ExternalInput").ap()
    wq_d = nc.dram_tensor("wq", [128, N_DC, 128], BF, kind="ExternalInput").ap()
    wk_d = nc.dram_tensor("wk", [128, N_DC, 128], BF, kind="ExternalInput").ap()
    wv_d = nc.dram_tensor("wv", [128, N_DC, 128], BF, kind="ExternalInput").ap()
    bq_d = nc.dram_tensor("bq", [128, 1], F32, kind="ExternalInput").ap()
    bk_d = nc.dram_tensor("bk", [128, 1], F32, kind="ExternalInput").ap()
    bv4_d = nc.dram_tensor("bv4", [1, 512], F32, kind="ExternalInput").ap()
    tri_d = nc.dram_tensor("tri", [128, 128], BF, kind="ExternalInput").ap()
    wout_d = nc.dram_tensor("wout", [128, N_DC, D], BF, kind="ExternalInput").ap()
    bout_d = nc.dram_tensor("bout", [1, D], F32, kind="ExternalInput").ap()
    out_d = nc.dram_tensor("out", [512, D], F32, kind="ExternalOutput").ap()

    y_part = nc.dram_tensor("y_part", [8, 128, SC], BF)
    y_all = nc.dram_tensor("y_all", [8, 128, SC], BF)
    z_bounce = nc.dram_tensor("z_bounce", [4, 1, SC], BF)
    cc_warm_in = nc.dram_tensor("cc_warm_in", [8, 1, 2], BF)
    cc_warm_out = nc.dram_tensor("cc_warm_out", [8, 1, 2], BF)
    if dbg:
        dbg_qT = nc.dram_tensor("dbg_qT", [B, 64, 2, S], BF, kind="ExternalOutput").ap()
        dbg_kT = nc.dram_tensor("dbg_kT", [B, 64, 2, S], BF, kind="ExternalOutput").ap()
        dbg_v = nc.dram_tensor("dbg_v", [B, 128, N_SB * 2 * 128], BF, kind="ExternalOutput").ap()
        dbg_z = nc.dram_tensor("dbg_z", [B, 4, 2, 1, SC], mybir.dt.float32, kind="ExternalOutput").ap()
        dbg_ex = nc.dram_tensor("dbg_ex", [B, 4, 2, 128, SC], BF, kind="ExternalOutput").ap()
        dbg_yp = nc.dram_tensor("dbg_yp", [8, 128, SC], BF, kind="ExternalOutput").ap()

    with tile.TileContext(nc) as tc, contextlib.ExitStack() as ctx:
        const = ctx.enter_context(tc.tile_pool(name="const", bufs=1))
        xt_pool = ctx.enter_context(tc.tile_pool(name="xt", bufs=18))
        qk_pool = ctx.enter_context(tc.tile_pool(name="qk", bufs=2))
        v_pool = ctx.enter_context(tc.tile_pool(name="vp", bufs=2))
        exp_pool = ctx.enter_context(tc.tile_pool(name="expp", bufs=6))
        yn_pool = ctx.enter_context(tc.tile_pool(name="yn", bufs=3))
        z_pool = ctx.enter_context(tc.tile_pool(name="zp", bufs=2))
        yg_pool = ctx.enter_context(tc.tile_pool(name="yg", bufs=1))
        outs_pool = ctx.enter_context(tc.tile_pool(name="outs", bufs=3))
        psum = ctx.enter_context(tc.tile_pool(name="psum", bufs=1, space="PSUM"))

        # ---- early constants (needed by QKV/attention) ----
        wq_sb = const.tile([128, N_DC, 128], BF)
        nc.sync.dma_start(out=wq_sb, in_=wq_d)
        wk_sb = const.tile([128, N_DC, 128], BF)
        nc.sync.dma_start(out=wk_sb, in_=wk_d)
        wv_sb = const.tile([128, N_DC, 128], BF)
        nc.sync.dma_start(out=wv_sb, in_=wv_d)
        bq_sb = const.tile([128, 1], F32)
        nc.sync.dma_start(out=bq_sb, in_=bq_d)
        bk_sb = const.tile([128, 1], F32)
        nc.sync.dma_start(out=bk_sb, in_=bk_d)
        bv4_bc = const.tile([128, 512], F32)
        nc.sync.dma_start(out=bv4_bc, in_=bv4_d.to_broadcast([128, 512]))
        tri_sb = const.tile([128, 128], BF)
        nc.sync.dma_start(out=tri_sb, in_=tri_d)

        # q/k/v tiles padded to full 128-partition / 128-column matmul shapes:
        # half-array matmuls (64-deep scores, 65-wide AV) keep the PE at the
        # slow p-state (~2x slower). The pad regions are zeroed once here and
        # never written again (bias-adds only touch the live regions), so the
        # zero-padding survives the per-batch reuse.
        qkT_bufs = []
        for bi in range(B):
            qTb = qk_pool.tile([128, 2, S], BF, tag=f"qT{bi}", bufs=1)
            kTb = qk_pool.tile([128, 2, S], BF, tag=f"kT{bi}", bufs=1)
            nc.gpsimd.memset(qTb[64:128, :, :], 0.0)
            nc.gpsimd.memset(kTb[64:128, :, :], 0.0)
            qkT_bufs.append((qTb, kTb))
        v_bufs = []
        for bi in range(B):
            v_sbb = v_pool.tile([128, N_SB, 2, 128], BF, tag=f"v{bi}", bufs=1)
            nc.gpsimd.memset(v_sbb[:, :, :, 65:128], 0.0)
            nc.gpsimd.memset(v_sbb[:, :, :, 64:65], 1.0)
            v_bufs.append(v_sbb)

        for b in range(B):
            # ---- QKV projection for batch b (2 heads) ----
            qT, kT = qkT_bufs[b]
            v_sb = v_bufs[b]
            for sc in range(N_SC):
                xts = []
                for dc in range(N_DC):
                    xt = xt_pool.tile([128, SC], BF)
                    nc.sync.dma_start(out=xt, in_=xt_d[b, sc, dc])
                    xts.append(xt)
                psq = psum.tile([128, SC], F32, tag="qkv", bufs=1)
                for dc in range(N_DC):
                    nc.tensor.matmul(
                        psq, wq_sb[:, dc, :], xts[dc],
                        start=(dc == 0), stop=(dc == N_DC - 1),
                    )
                for h in range(2):
                    nc.vector.tensor_scalar_add(
                        out=qT[0:64, h, ds(sc * SC, SC)],
                        in0=psq[ds(64 * h, 64), :],
                        scalar1=bq_sb[ds(64 * h, 64), :],
                    )

                psk = psum.tile([128, SC], F32, tag="qkv", bufs=1)
                for dc in range(N_DC):
                    nc.tensor.matmul(
                        psk, wk_sb[:, dc, :], xts[dc],
                        start=(dc == 0), stop=(dc == N_DC - 1),
                    )
                for h in range(2):
                    nc.vector.tensor_scalar_add(
                        out=kT[0:64, h, ds(sc * SC, SC)],
                        in0=psk[ds(64 * h, 64), :],
                        scalar1=bk_sb[ds(64 * h, 64), :],
                    )

                psv = psum.tile([128, 4, 128], F32, tag="psv", bufs=1)
                for j4 in range(4):
                    for dc in range(N_DC):
                        nc.tensor.matmul(
                            psv[:, j4, :],
                            xts[dc][:, ds(j4 * 128, 128)],
                            wv_sb[:, dc, :],
                            start=(dc == 0), stop=(dc == N_DC - 1),
                            skip_group_check=True,
                        )
                nc.vector.tensor_add(
                    out=v_sb[:, ds(4 * sc, 4), :, 0:64],
                    in0=psv.rearrange("p j (h e) -> p j h e", h=2),
                    in1=bv4_bc.rearrange("p (j h e) -> p j h e", j=4, h=2),
                )

            # ---- attention for batch b ----
            for qc in range(N_SC):
                ntb = 4 * qc + 4
                psys = [
                    psum.tile([128, SC], F32, tag="psy", bufs=2, name=f"psy{h}")
                    for h in range(2)
                ]
                exs = {}

                def issue_score(tb, qc=qc, exs=exs):
                    j = tb - 4 * qc  # >= 0: diagonal block index
                    off = 128 * j if j >= 0 else 0
                    w = SC - off
                    for h in range(2):
                        pss = psum.tile([128, SC], F32, tag="pss", bufs=4)
                        nc.tensor.matmul(
                            pss[:, off:],
                            kT[:, h, ds(tb * 128, 128)],
                            qT[:, h, ds(qc * SC + off, w)],
                            start=True, stop=True,
                        )
                        ex = exp_pool.tile([128, SC], BF)
                        nc.scalar.activation(
                            out=ex[:, off:], in_=pss[:, off:],
                            func=mybir.ActivationFunctionType.Exp,
                            scale=0.125,
                        )
                        if j >= 0:
                            nc.vector.tensor_mul(
                                out=ex[:, ds(off, 128)],
                                in0=ex[:, ds(off, 128)],
                                in1=tri_sb,
                            )
                        if dbg and tb == 0:
                            nc.sync.dma_start(out=dbg_ex[b, qc, h], in_=ex)
                        exs[(tb, h)] = (ex, off)

                issue_score(0)
                if ntb > 1:
                    issue_score(1)
                for tb in range(ntb):
                    if tb + 2 < ntb:
                        issue_score(tb + 2)
                    for h in range(2):
                        ex, off = exs.pop((tb, h))
                        nc.tensor.matmul(
                            psys[h][:, off:],
                            v_sb[:, tb, h, :],
                            ex[:, off:],
                            start=(tb == 0), stop=(tb == ntb - 1),
                            skip_group_check=True,
                        )
                for h in range(2):
                    psy = psys[h]
                    # on the final chunk, psum->sbuf copies go to the (idle)
                    # scalar engine so the tail drain isn't vector-serial
                    if qc == 3:
                        def _cp(out, in_):
                            nc.scalar.copy(out=out, in_=in_)
                    else:
                        def _cp(out, in_):
                            nc.vector.tensor_copy(out=out, in_=in_)
                    yraw = yn_pool.tile([64, SC], BF, tag=f"yraw{h}")
                    _cp(out=yraw, in_=psy[0:64, :])
                    zs = z_pool.tile([1, SC], F32, tag=f"zs{h}")
                    _cp(out=zs, in_=psy[64:65, :])
                    zr = z_pool.tile([1, SC], F32, tag=f"zr{h}")
                    nc.vector.reciprocal_approx_fast(out=zr, in_=zs)
                    if dbg:
                        nc.sync.dma_start(out=dbg_z[b, qc, h], in_=zr)
                    zb = z_pool.tile([1, SC], BF, tag=f"zb{h}")
                    nc.vector.tensor_copy(out=zb, in_=zr)
                    zd = z_bounce.ap()[(2 * qc + h) % 4]
                    nc.sync.dma_start(out=zd, in_=zb)
                    zbb = z_pool.tile([64, SC], BF, tag=f"zbb{h}")
                    nc.sync.dma_start(out=zbb, in_=zd.to_broadcast([64, SC]))
                    yts = yn_pool.tile([64, SC], BF, tag=f"yts{h}")
                    nc.vector.tensor_mul(out=yts, in0=yraw, in1=zbb)
                    nc.sync.dma_start(
                        out=y_part.ap()[b * 4 + qc, ds(64 * h, 64), :], in_=yts
                    )

            if b == 0:
                # tiny warm-up AllToAll: pre-pays the collective ring/DGE
                # setup so the real one at the end launches fast
                nc.gpsimd.collective_compute(
                    "AllToAll",
                    mybir.AluOpType.bypass,
                    replica_groups=[list(range(N_CORES))],
                    ins=[cc_warm_in.ap()],
                    outs=[cc_warm_out.ap()],
                )

            if dbg:
                nc.sync.dma_start(out=dbg_qT[b], in_=qT)
                nc.sync.dma_start(out=dbg_kT[b], in_=kT)
                nc.sync.dma_start(
                    out=dbg_v[b], in_=v_sb.rearrange("p a b c -> p (a b c)")
                )

        if dbg:
            nc.sync.dma_start(out=dbg_yp, in_=y_part.ap())

        # ---- late constants (output projection) ----
        wout_sb = const.tile([128, N_DC, D], BF)
        nc.sync.dma_start(out=wout_sb, in_=wout_d)
        bout_bc = const.tile([128, D], F32)
        nc.sync.dma_start(out=bout_bc, in_=bout_d.to_broadcast([128, D]))

        # ---- reshard: head-split -> row-split ----
        nc.gpsimd.collective_compute(
            "AllToAll",
            mybir.AluOpType.bypass,
            replica_groups=[list(range(N_CORES))],
            ins=[y_part.ap()],
            outs=[y_all.ap()],
        )

        # ---- output projection for this core's 512 rows ----
        ygs = []
        for ec in range(8):
            yg = yg_pool.tile([128, SC], BF, tag=f"yg{ec}")
            nc.sync.dma_start(out=yg, in_=y_all.ap()[ec])
            ygs.append(yg)
        for sb in range(4):
            for ch in range(2):
                pso = psum.tile([128, SC], F32, tag="pss", bufs=4)
                for ec in range(8):
                    nc.tensor.matmul(
                        pso,
                        ygs[ec][:, ds(sb * 128, 128)],
                        wout_sb[:, ec, ds(ch * SC, SC)],
                        start=(ec == 0), stop=(ec == 7),
                    )
                ot = outs_pool.tile([128, SC], F32)
                nc.vector.tensor_add(
                    out=ot, in0=pso, in1=bout_bc[:, ds(ch * SC, SC)]
                )
                nc.sync.dma_start(
                    out=out_d[ds(sb * 128, 128), ds(ch * SC, SC)], in_=ot
                )

    nc.compile()
    return nc


_NC_CACHE = None


def _get_program():
    global _NC_CACHE
    if _NC_CACHE is None:
        _NC_CACHE = _build_program()
    return _NC_CACHE


def make_in_maps(x, Wqkv, bqkv, Wout, bout):
    x = np.asarray(x, dtype=np.float32)
    Wqkv = np.asarray(Wqkv, dtype=np.float32)
    bqkv = np.asarray(bqkv, dtype=np.float32)
    Wout = np.asarray(Wout, dtype=np.float32)
    bout = np.asarray(bout, dtype=np.float32)

    # xt[b, sc, dc, p, col] = x[b, sc*512+col, 128*dc+p]
    xt = np.ascontiguousarray(
        x.reshape(B, N_SC, SC, N_DC, 128).transpose(0, 1, 3, 4, 2)
    ).astype(NPBF)
    wout_t = np.ascontiguousarray(
        Wout.reshape(N_DC, 128, D).transpose(1, 0, 2)
    ).astype(NPBF)
    bout2 = np.ascontiguousarray(bout.reshape(1, D))
    tri = np.triu(np.ones((128, 128), dtype=np.float32)).astype(NPBF)

    def wslice(lo, hi, c):
        h0, h1 = 2 * c, 2 * c + 1
        w = np.concatenate([Wqkv[h0, :, lo:hi], Wqkv[h1, :, lo:hi]], axis=1)
        return np.ascontiguousarray(
            w.reshape(N_DC, 128, 128).transpose(1, 0, 2)
        ).astype(NPBF)

    in_maps = []
    for c in range(N_CORES):
        h0, h1 = 2 * c, 2 * c + 1
        bq = np.concatenate([bqkv[h0, 0:64], bqkv[h1, 0:64]]).reshape(128, 1)
        bk = np.concatenate([bqkv[h0, 64:128], bqkv[h1, 64:128]]).reshape(128, 1)
        bvp = np.concatenate([bqkv[h0, 128:192], bqkv[h1, 128:192]])
        bv4 = np.tile(bvp, 4).reshape(1, 512)
        in_maps.append(
            {
                "xt": xt,
                "wq": wslice(0, 64, c),
                "wk": wslice(64, 128, c),
                "wv": wslice(128, 192, c),
                "bq": np.ascontiguousarray(bq),
                "bk": np.ascontiguousarray(bk),
                "bv4": np.ascontiguousarray(bv4),
                "tri": tri,
                "wout": wout_t,
                "bout": bout2,
            }
        )
    return in_maps


def assemble(results):
    full = np.empty((N_CORES * 512, D), dtype=np.float32)
    for c in range(N_CORES):
        full[512 * c : 512 * (c + 1)] = results[c]["out"]
    return full.reshape(B, S, D)


def _install_ntff_hook():
    """The agent image's antenv lacks axon_hooks; provide it so
    run_bass_kernel_spmd(trace=True) can NTFF-profile via libaxon."""
    if "antenv.axon_hooks" in sys.modules:
        return
    so_path = "/opt/axon/libaxon_pjrt.so"
    try:
        lib = ctypes.CDLL(so_path)
        lib.axon_start_nrt_profile.argtypes = [
            ctypes.POINTER(ctypes.c_int64),
            ctypes.c_size_t,
        ]
        lib.axon_start_nrt_profile.restype = ctypes.c_int64
        lib.axon_stop_nrt_profile.argtypes = [ctypes.c_char_p]
        lib.axon_stop_nrt_profile.restype = ctypes.c_int64
    except (OSError, AttributeError):
        return

    @contextlib.contextmanager
    def _hook(output_dir, device_ids):
        import jax

        jax.devices()
        if device_ids:
            ids = (ctypes.c_int64 * len(device_ids))(*device_ids)
            rc = lib.axon_start_nrt_profile(ids, len(device_ids))
        else:
            rc = lib.axon_start_nrt_profile(None, 0)
        if rc != 0:
            raise RuntimeError(f"axon_start_nrt_profile rc={rc}")
        try:
            yield
        finally:
            n = lib.axon_stop_nrt_profile(str(output_dir).encode())
            if n < 0:
                raise RuntimeError(f"axon_stop_nrt_profile rc={n}")

    mod = types.ModuleType("antenv.axon_hooks")
    mod.get_axon_ntff_profile_hook = lambda: _hook
    mod.set_axon_ntff_profile_hook = lambda h: None
    sys.modules["antenv.axon_hooks"] = mod


def run(inputs, trace=False):
    """Run on the 8 NeuronCores. Returns (output, BassKernelResults)."""
    from concourse.bass_utils import run_bass_kernel_spmd

    if trace:
        _install_ntff_hook()
    nc = _get_program()
    in_maps = make_in_maps(**inputs)
    res = run_bass_kernel_spmd(
        nc, in_maps, core_ids=list(range(N_CORES)), trace=trace
    )
    return assemble(res.results), res


def kernel(x, Wqkv, bqkv, Wout, bout):
    out, _ = run(
        {"x": x, "Wqkv": Wqkv, "bqkv": bqkv, "Wout": Wout, "bout": bout},
        trace=False,
    )
    return out
